# revision 24
# baseline (speedup 1.0000x reference)
"""Trainium2 Bass kernel for DirectConv2D (3x3 VALID, NCHW/OIHW).

Problem: x [32, 256, 56, 56] int32 (values 0..7 after clip),
         weight [256, 256, 3, 3] fp32 (small non-negative ints 0..6)
         -> out [32, 256, 54, 54] fp32.

Strategy:
 - Data-parallel across 8 NeuronCores: 4 images per core, weight replicated.
 - Conv decomposed into 9 shifted matmuls (one per kernel tap) accumulated
   in PSUM; contraction over the 256 input channels.
 - Inputs are tiny non-negative integers, so fp8-e4m3 matmuls are exact
   (products <= 42, fp32 PSUM accumulation). DoubleRow perf mode contracts
   all 256 input channels (2 x 128-partition k-tiles) per matmul.
 - Activations live in SBUF with the two 128-channel chunks INTERLEAVED
   at adjacent bytes: [128 part, pix 3140, chunk 2] (56*56=3136 pixels +
   4 pad so every tile can read a full 504-wide window). The DoubleRow
   moving AP then steps 2 bytes per pixel with the chunk pair contiguous,
   so every tap's base byte is EVEN — avoiding the +7.5ns/matmul odd-base
   issue penalty kw=1 taps would otherwise pay — and each image loads in
   a single DMA. Output computed in tiles of 9 rows x 56 cols = 504 <=
   512 (one PSUM bank); only the 54 valid cols per row are stored.
 - Output staged/stored as fp16 (exact ints up to ~27k round to <=8 abs,
   ~3e-4 rel — far within tolerance) halving store traffic; host upcasts.
 - Head: tiny warm-up memsets on DVE (free + boots early), critical input
   DMAs issued first across the sync AND scalar rings in parallel, small
   warm-up (4 junk MMs) only to bridge until the first real data lands;
   real matmuls then start immediately (cold-rate real work beats junk).
 - Input loads are HBM-bandwidth-bound across the 8 cores, so total load
   bytes are kept minimal (no odd-shifted x duplicates: the ~7.5ns/matmul
   odd-base penalty on kw=1 taps costs ~1.1us total, far less than the
   ~6.7us of PE stall the extra 2.4MB of loads caused).
"""

import sys

sys.path.insert(0, "/opt/trn_rl_repo")

import ml_dtypes
import numpy as np

N_CORES = 8
IMGS = 4  # images per core
H = W = 56
OH = OW = 54
PIX = H * W  # 3136
PIXP = PIX + 4  # padded so kh=2,kw=2 window of width 504 stays in-bounds
ROWS_PER_TILE = 9
N_TILE = ROWS_PER_TILE * W  # 504 (<= 512 fp32 PSUM bank)
N_ROWTILES = OH // ROWS_PER_TILE  # 6

_PROGRAM_CACHE = {}


def _build_program(mode="fp8dr"):
    import concourse.bacc as bacc
    import concourse.mybir as mybir
    import concourse.tile as tile

    nc = bacc.Bacc(
        "TRN2",
        target_bir_lowering=False,
        debug=False,
        enable_asserts=False,
        num_devices=N_CORES,
    )
    dt8 = mybir.dt.float8e4
    dtb = mybir.dt.bfloat16
    dt_in = dt8 if mode == "fp8dr" else dtb
    dt_out = mybir.dt.float16

    x_d = nc.dram_tensor("x_sb", [128, IMGS, PIXP, 2], dt_in, kind="ExternalInput").ap()
    w_d = nc.dram_tensor("w_sb", [128, 2, 9, 2, 128], dt_in, kind="ExternalInput").ap()
    out_d = nc.dram_tensor(
        "out", [IMGS, 256, OH, OW], dt_out, kind="ExternalOutput"
    ).ap()

    NT486 = ROWS_PER_TILE * OW  # 486 output pixels per row tile
    X0A_END = 1232  # image-0 leading tile: rows 0..21 (covers row tiles 0,1)
    X0M_BASE, X0M_END = 1008, 2140  # image-0 middle tile (row tiles 2,3)
    X0Z_BASE = 2016  # image-0 trailing tile (row tiles 4,5)

    with tile.TileContext(nc) as tc:
        with (
            tc.tile_pool(name="const", bufs=1) as const_pool,
            tc.tile_pool(name="psum", bufs=8, space="PSUM") as psum_pool,
            tc.tile_pool(name="outs", bufs=3) as out_pool,
        ):
            # PE warm-up on scratch: a handful of matmuls bridging the gap
            # between engine start and the first real input data landing, so
            # HAM un-throttling begins ASAP. Results are never read (next
            # user of the PSUM slot starts with start=True). Memsets go on
            # DVE (idle until the first PSUM evacuation ~15us in) so the
            # warm-up's only dependency clears within ~100ns of engine start.
            w_warm = const_pool.tile([128, 2, 128], dt_in)
            x_warm = const_pool.tile([128, 2, 544], dt_in)
            if mode != "fp8dr":
                nc.vector.memset(w_warm, 0.0)
                nc.vector.memset(x_warm, 0.0)
            else:
                # tiles must have a writer to be allocated; a 2-byte memset
                # is enough and keeps the warm-up dependency nearly free
                nc.vector.memset(w_warm[:, 0, 0:2], 0.0)
                nc.vector.memset(x_warm[:, 0, 0:2], 0.0)
            # Under 8-core HBM contention the first input chunks land ~10us
            # in; junk MMs bridge the whole window — any PE idle gap before
            # the HAM SHORT window completes resets the un-throttle clock
            # (costs ~2-4us of half-rate), so over-bridging is the safe side.
            # Junk bridge: 3 full-width MMs (pipeline fill) then short ones,
            # so however late the first real data lands (10.5-12.6us spread
            # under 8-core HBM contention), the PE stays continuously busy
            # (a gap resets the HAM un-throttle window, ~2-4us of half-rate)
            # while the overrun past data-ready is at most ~0.2us.
            pt_warm = psum_pool.tile([128, NT486], mybir.dt.float32, tag="pt")
            warm_ns = [486] * 3 + [243] * 18
            for i, nw in enumerate(warm_ns):
                rhs_w = x_warm[:, :, 0:nw].rearrange(
                    "p c (r q) -> p c r q", q=81
                )
                if mode == "fp8dr":
                    nc.tensor.matmul(
                        pt_warm[:, 0:nw], w_warm, rhs_w,
                        start=(i == 0), stop=(i == len(warm_ns) - 1),
                        perf_mode=mybir.MatmulPerfMode.DoubleRow,
                    )
                else:
                    nc.tensor.matmul(
                        pt_warm[:, 0:nw], w_warm[:, 0], rhs_w[:, 0],
                        start=(i == 0), stop=(i == len(warm_ns) - 1),
                    )

            # Weights split into three tiles so dependency tracking (which is
            # per-tile) lets the first accumulation group start as soon as the
            # small taps-0..2 chunk lands, instead of the whole 4.6KB/part.
            wt0 = const_pool.tile([128, 9, 2, 128], dt_in)  # oc0 all taps
            wt1 = const_pool.tile([128, 9, 2, 128], dt_in)  # oc1 all taps
            # Per-image x tiles so matmul deps only cover the image they
            # read (dependency tracking is per-tile). Chunk-interleaved:
            # [128, pix, 2].
            xt0a = const_pool.tile([128, X0A_END, 2], dt_in)
            xt0m = const_pool.tile([128, X0M_END - X0M_BASE, 2], dt_in)
            xt0z = const_pool.tile([128, PIXP - X0Z_BASE, 2], dt_in)
            xts = [None] + [
                const_pool.tile([128, PIXP, 2], dt_in, name=f"xt{n}", tag=f"xt{n}")
                for n in (1, 2, 3)
            ]
            # dma_start issue costs ~600ns serialized per sequencer, so the
            # first-needed bytes go at slot 0 of BOTH hw rings in parallel
            # (image-0 lead on sync, its first weight taps on scalar);
            # everything later is ordered by first-use time.
            nc.sync.dma_start(out=xt0a[:, 0:620], in_=x_d[:, 0, 0:620])
            nc.sync.dma_start(out=xt0a[:, 620:], in_=x_d[:, 0, 620:X0A_END])
            nc.sync.dma_start(out=xt0m, in_=x_d[:, 0, X0M_BASE:X0M_END])
            nc.sync.dma_start(out=wt1, in_=w_d[:, 1])
            nc.sync.dma_start(out=xts[1], in_=x_d[:, 1])
            nc.sync.dma_start(out=xts[3], in_=x_d[:, 3])
            nc.scalar.dma_start(out=wt0, in_=w_d[:, 0])
            nc.scalar.dma_start(out=xt0z, in_=x_d[:, 0, X0Z_BASE:])
            nc.scalar.dma_start(out=xts[2], in_=x_d[:, 2])

            def x_src(n, t):
                """(x tile, pixel base) holding rows needed by row tile t."""
                if n == 0:
                    if t < 2:
                        return xt0a, 0
                    if t < 4:
                        return xt0m, X0M_BASE
                    return xt0z, X0Z_BASE
                return xts[n], 0

            def w_sel(oc, k):
                """Stationary weight AP [128, 2, 128] for (oc, tap k)."""
                return (wt0 if oc == 0 else wt1)[:, k]

            # Last block gets a small trailing row-tile so the final
            # PSUM-evacuate -> store -> HBM-write-receipt chain (which the
            # exit barrier serializes on) covers only 4 rows.
            TILES = [(t * ROWS_PER_TILE, ROWS_PER_TILE) for t in range(N_ROWTILES)]
            TILES_LAST = TILES[:5] + [(45, 5), (50, 4)]

            for n in range(IMGS):
                for oc in range(2):
                    last_block = n == IMGS - 1 and oc == 1
                    tiles = TILES_LAST if last_block else TILES
                    # staging for a full (n, oc) output block: dense 54x54
                    # rows so stores move 5.8KB-contiguous lines/partition.
                    ot = out_pool.tile([128, OH * OW], dt_out)
                    for t, (h0, rows) in enumerate(tiles):
                        nt_in = rows * W
                        nt_out = rows * OW
                        ob = h0 * OW  # fp16 staging offset of this tile
                        xsrc, xbase = x_src(n, min(t, 5))
                        pt = psum_pool.tile([128, nt_out], mybir.dt.float32)
                        k = 0
                        for kh in range(3):
                            for kw in range(3):
                                xs, xb = xsrc, xbase
                                off = (h0 + kh) * W + kw - xb
                                # strided moving AP skips the 2 junk cols per
                                # row: [128, 2 chunks (stride 1B), rows
                                # (stride 112B), 54 cols (stride 2B)] — base
                                # byte 2*off is always even.
                                if mode == "fp8dr":
                                    rhs = xs[:, off : off + nt_in, :].rearrange(
                                        "p (r q) c -> p c r q", q=W
                                    )[:, :, :, 0:OW]
                                    nc.tensor.matmul(
                                        pt,
                                        w_sel(oc, k),
                                        rhs,
                                        start=(k == 0),
                                        stop=(k == 8),
                                        perf_mode=mybir.MatmulPerfMode.DoubleRow,
                                    )
                                else:
                                    for c in range(2):
                                        rhs = xs[:, off : off + nt_in, c].rearrange(
                                            "p (r q) -> p r q", q=W
                                        )[:, :, 0:OW]
                                        nc.tensor.matmul(
                                            pt,
                                            w_sel(oc, k)[:, c],
                                            rhs,
                                            start=(k == 0 and c == 0),
                                            stop=(k == 8 and c == 1),
                                        )
                                k += 1
                        nc.vector.tensor_copy(
                            out=ot[:, ob : ob + nt_out], in_=pt
                        )
                        if last_block:
                            # fine-grained stores on the final block: pairs
                            # early, singles at the end so the final store
                            # (and its HBM write receipt, which the exit
                            # barrier waits on) covers only 4 rows. The
                            # 5-row and 4-row pieces go on different rings.
                            if t in (1, 3):
                                nc.sync.dma_start(
                                    out=out_d[n, oc * 128 : (oc + 1) * 128,
                                              h0 - ROWS_PER_TILE : h0 + rows, :],
                                    in_=ot[:, ob - ROWS_PER_TILE * OW : ob + nt_out]
                                    .rearrange("p (h w) -> p h w", w=OW),
                                )
                            elif t >= 4:
                                ring = nc.scalar if t == len(tiles) - 1 else nc.sync
                                ring.dma_start(
                                    out=out_d[n, oc * 128 : (oc + 1) * 128,
                                              h0 : h0 + rows, :],
                                    in_=ot[:, ob : ob + nt_out].rearrange(
                                        "p (h w) -> p h w", w=OW
                                    ),
                                )
                    if not last_block:
                        nc.sync.dma_start(
                            out=out_d[n, oc * 128 : (oc + 1) * 128, :, :],
                            in_=ot.rearrange("p (h w) -> p h w", w=OW),
                        )
    nc.compile()
    return nc


def get_program(mode="fp8dr"):
    if mode not in _PROGRAM_CACHE:
        _PROGRAM_CACHE[mode] = _build_program(mode)
    return _PROGRAM_CACHE[mode]


def _np_dtype(mode):
    return ml_dtypes.float8_e4m3 if mode == "fp8dr" else ml_dtypes.bfloat16


def prep_weight(weight, mode="fp8dr"):
    """weight [256, 256, 3, 3] OIHW fp32 -> w_sb [128 ki, 2 oc, 9 tap, 2 c, 128 m]."""
    wq = weight.astype(np.int32).astype(np.float32)
    wq = wq.reshape(2, 128, 2, 128, 3, 3)  # [oc, m, c, ki, kh, kw]
    w_sb = np.ascontiguousarray(wq.transpose(3, 0, 4, 5, 2, 1))  # [ki, oc, kh, kw, c, m]
    w_sb = w_sb.reshape(128, 2, 9, 2, 128)
    return w_sb.astype(_np_dtype(mode))


def prep_x_core(x_core, mode="fp8dr"):
    """x_core [IMGS, 256, 56, 56] int32 -> x_sb [128 ki, IMGS, PIXP, 2 c]."""
    xq = np.clip(x_core.astype(np.int32), 0, 7).astype(np.float32)
    xq = xq.reshape(IMGS, 2, 128, PIX)  # [n, c, ki, pix]
    x_sb = np.zeros((128, IMGS, PIXP, 2), np.float32)
    x_sb[:, :, :PIX, :] = xq.transpose(2, 0, 3, 1)
    return x_sb.astype(_np_dtype(mode))


def make_in_maps(x, weight, mode="fp8dr"):
    w_sb = prep_weight(weight, mode)
    return [
        {"x_sb": prep_x_core(x[c * IMGS : (c + 1) * IMGS], mode), "w_sb": w_sb}
        for c in range(N_CORES)
    ]


def kernel(x, weight):
    import time

    from concourse.bass_utils import run_bass_kernel_spmd

    mode = "fp8dr"
    nc = get_program(mode)
    in_maps = make_in_maps(np.asarray(x), np.asarray(weight), mode)
    last_err = None
    for attempt in range(3):
        try:
            res = run_bass_kernel_spmd(nc, in_maps, list(range(N_CORES)))
            break
        except Exception as e:  # transient NRT_EXEC_UNIT_UNRECOVERABLE flakes
            last_err = e
            time.sleep(2.0)
    else:
        raise last_err
    return np.concatenate(
        [res.results[c]["out"] for c in range(N_CORES)], axis=0
    ).astype(np.float32)


# revision 25
# speedup vs baseline: 1.1937x; 1.1937x over previous
"""Trainium2 Bass kernel for DirectConv2D (3x3 VALID, NCHW/OIHW).

Problem: x [32, 256, 56, 56] int32 (values 0..7 after clip),
         weight [256, 256, 3, 3] fp32 (small non-negative ints 0..6)
         -> out [32, 256, 54, 54] fp32.

Strategy:
 - Data-parallel across 8 NeuronCores: 4 images per core, weight replicated.
 - Conv decomposed into 9 shifted matmuls (one per kernel tap) accumulated
   in PSUM; contraction over the 256 input channels.
 - Inputs are tiny non-negative integers, so fp8-e4m3 matmuls are exact
   (products <= 42, fp32 PSUM accumulation). DoubleRow perf mode contracts
   all 256 input channels (2 x 128-partition k-tiles) per matmul.
 - Activations live in SBUF with the two 128-channel chunks INTERLEAVED
   at adjacent bytes: [128 part, pix 3140, chunk 2] (56*56=3136 pixels +
   4 pad so every tile can read a full 504-wide window). The DoubleRow
   moving AP then steps 2 bytes per pixel with the chunk pair contiguous,
   so every tap's base byte is EVEN — avoiding the +7.5ns/matmul odd-base
   issue penalty kw=1 taps would otherwise pay — and each image loads in
   a single DMA. Output computed in tiles of 9 rows x 56 cols = 504 <=
   512 (one PSUM bank); only the 54 valid cols per row are stored.
 - Output staged/stored as fp16 (exact ints up to ~27k round to <=8 abs,
   ~3e-4 rel — far within tolerance) halving store traffic; host upcasts.
 - Head: tiny warm-up memsets on DVE (free + boots early), critical input
   DMAs issued first across the sync AND scalar rings in parallel, and a
   fine-grained junk-MM bridge (3x486 + 18x243 cols) keeps the PE busy
   until the first real data lands (10.5-12.6us under 8-core HBM
   contention) — any PE idle gap resets the HAM un-throttle window
   (~2-4us of half-rate), while bridge overrun costs only ~0.2us steps.
 - Input loads are HBM-bandwidth-bound across the 8 cores, so total load
   bytes are kept minimal (no odd-shifted x duplicates), and the oc0
   weight block loads as ONE tile so the first accumulation group waits
   on a single DMA completion (a taps-0-2/3-8 split stalled mid-group on
   unlucky cores).
 - Tail: last block tiled 9,9,9,9,9,5,4 rows so the final PSUM-evacuate
   -> store -> HBM-write-receipt chain (which the exit barrier serializes
   on) covers only 4 rows, with the 5/4-row stores on different rings.
"""

import sys

sys.path.insert(0, "/opt/trn_rl_repo")

import ml_dtypes
import numpy as np

N_CORES = 8
IMGS = 4  # images per core
H = W = 56
OH = OW = 54
PIX = H * W  # 3136
PIXP = PIX + 4  # padded so kh=2,kw=2 window of width 504 stays in-bounds
ROWS_PER_TILE = 9
N_TILE = ROWS_PER_TILE * W  # 504 (<= 512 fp32 PSUM bank)
N_ROWTILES = OH // ROWS_PER_TILE  # 6

_PROGRAM_CACHE = {}


def _build_program(mode="fp8dr"):
    import concourse.bacc as bacc
    import concourse.mybir as mybir
    import concourse.tile as tile

    nc = bacc.Bacc(
        "TRN2",
        target_bir_lowering=False,
        debug=False,
        enable_asserts=False,
        num_devices=N_CORES,
    )
    dt8 = mybir.dt.float8e4
    dtb = mybir.dt.bfloat16
    dt_in = dt8 if mode == "fp8dr" else dtb
    dt_out = mybir.dt.float16

    x_d = nc.dram_tensor("x_sb", [128, IMGS, PIXP, 2], dt_in, kind="ExternalInput").ap()
    w_d = nc.dram_tensor("w_sb", [128, 2, 9, 2, 128], dt_in, kind="ExternalInput").ap()
    out_d = nc.dram_tensor(
        "out", [IMGS, 256, OH, OW], dt_out, kind="ExternalOutput"
    ).ap()

    NT486 = ROWS_PER_TILE * OW  # 486 output pixels per row tile
    X0A_END = 1232  # image-0 leading tile: rows 0..21 (covers row tiles 0,1)
    X0M_BASE, X0M_END = 1008, 2140  # image-0 middle tile (row tiles 2,3)
    X0Z_BASE = 2016  # image-0 trailing tile (row tiles 4,5)

    with tile.TileContext(nc) as tc:
        with (
            tc.tile_pool(name="const", bufs=1) as const_pool,
            tc.tile_pool(name="psum", bufs=8, space="PSUM") as psum_pool,
            tc.tile_pool(name="outs", bufs=3) as out_pool,
        ):
            # PE warm-up on scratch: a handful of matmuls bridging the gap
            # between engine start and the first real input data landing, so
            # HAM un-throttling begins ASAP. Results are never read (next
            # user of the PSUM slot starts with start=True). Memsets go on
            # DVE (idle until the first PSUM evacuation ~15us in) so the
            # warm-up's only dependency clears within ~100ns of engine start.
            w_warm = const_pool.tile([128, 2, 128], dt_in)
            x_warm = const_pool.tile([128, 2, 544], dt_in)
            if mode != "fp8dr":
                nc.vector.memset(w_warm, 0.0)
                nc.vector.memset(x_warm, 0.0)
            else:
                # tiles must have a writer to be allocated; a 2-byte memset
                # is enough and keeps the warm-up dependency nearly free
                nc.vector.memset(w_warm[:, 0, 0:2], 0.0)
                nc.vector.memset(x_warm[:, 0, 0:2], 0.0)
            # Under 8-core HBM contention the first input chunks land ~10us
            # in; junk MMs bridge the whole window — any PE idle gap before
            # the HAM SHORT window completes resets the un-throttle clock
            # (costs ~2-4us of half-rate), so over-bridging is the safe side.
            # Junk bridge: 3 full-width MMs (pipeline fill) then short ones,
            # so however late the first real data lands (10.5-12.6us spread
            # under 8-core HBM contention), the PE stays continuously busy
            # (a gap resets the HAM un-throttle window, ~2-4us of half-rate)
            # while the overrun past data-ready is at most ~0.2us.
            pt_warm = psum_pool.tile([128, NT486], mybir.dt.float32, tag="pt")
            warm_ns = [486] * 3 + [243] * 18
            for i, nw in enumerate(warm_ns):
                rhs_w = x_warm[:, :, 0:nw].rearrange(
                    "p c (r q) -> p c r q", q=81
                )
                if mode == "fp8dr":
                    nc.tensor.matmul(
                        pt_warm[:, 0:nw], w_warm, rhs_w,
                        start=(i == 0), stop=(i == len(warm_ns) - 1),
                        perf_mode=mybir.MatmulPerfMode.DoubleRow,
                    )
                else:
                    nc.tensor.matmul(
                        pt_warm[:, 0:nw], w_warm[:, 0], rhs_w[:, 0],
                        start=(i == 0), stop=(i == len(warm_ns) - 1),
                    )

            # Weights split into three tiles so dependency tracking (which is
            # per-tile) lets the first accumulation group start as soon as the
            # small taps-0..2 chunk lands, instead of the whole 4.6KB/part.
            wt0 = const_pool.tile([128, 9, 2, 128], dt_in)  # oc0 all taps
            wt1 = const_pool.tile([128, 9, 2, 128], dt_in)  # oc1 all taps
            # Per-image x tiles so matmul deps only cover the image they
            # read (dependency tracking is per-tile). Chunk-interleaved:
            # [128, pix, 2].
            xt0a = const_pool.tile([128, X0A_END, 2], dt_in)
            xt0m = const_pool.tile([128, X0M_END - X0M_BASE, 2], dt_in)
            xt0z = const_pool.tile([128, PIXP - X0Z_BASE, 2], dt_in)
            xts = [None] + [
                const_pool.tile([128, PIXP, 2], dt_in, name=f"xt{n}", tag=f"xt{n}")
                for n in (1, 2, 3)
            ]
            # dma_start issue costs ~600ns serialized per sequencer, so the
            # first-needed bytes go at slot 0 of BOTH hw rings in parallel
            # (image-0 lead on sync, its first weight taps on scalar);
            # everything later is ordered by first-use time.
            nc.sync.dma_start(out=xt0a[:, 0:620], in_=x_d[:, 0, 0:620])
            nc.sync.dma_start(out=xt0a[:, 620:], in_=x_d[:, 0, 620:X0A_END])
            nc.sync.dma_start(out=xt0m, in_=x_d[:, 0, X0M_BASE:X0M_END])
            nc.sync.dma_start(out=wt1, in_=w_d[:, 1])
            nc.sync.dma_start(out=xts[1], in_=x_d[:, 1])
            nc.sync.dma_start(out=xts[3], in_=x_d[:, 3])
            nc.scalar.dma_start(out=wt0, in_=w_d[:, 0])
            nc.scalar.dma_start(out=xt0z, in_=x_d[:, 0, X0Z_BASE:])
            nc.scalar.dma_start(out=xts[2], in_=x_d[:, 2])

            def x_src(n, t):
                """(x tile, pixel base) holding rows needed by row tile t."""
                if n == 0:
                    if t < 2:
                        return xt0a, 0
                    if t < 4:
                        return xt0m, X0M_BASE
                    return xt0z, X0Z_BASE
                return xts[n], 0

            def w_sel(oc, k):
                """Stationary weight AP [128, 2, 128] for (oc, tap k)."""
                return (wt0 if oc == 0 else wt1)[:, k]

            # Last block gets a small trailing row-tile so the final
            # PSUM-evacuate -> store -> HBM-write-receipt chain (which the
            # exit barrier serializes on) covers only 4 rows.
            TILES = [(t * ROWS_PER_TILE, ROWS_PER_TILE) for t in range(N_ROWTILES)]
            TILES_LAST = TILES[:5] + [(45, 5), (50, 4)]

            for n in range(IMGS):
                for oc in range(2):
                    last_block = n == IMGS - 1 and oc == 1
                    tiles = TILES_LAST if last_block else TILES
                    # staging for a full (n, oc) output block: dense 54x54
                    # rows so stores move 5.8KB-contiguous lines/partition.
                    ot = out_pool.tile([128, OH * OW], dt_out)
                    for t, (h0, rows) in enumerate(tiles):
                        nt_in = rows * W
                        nt_out = rows * OW
                        ob = h0 * OW  # fp16 staging offset of this tile
                        xsrc, xbase = x_src(n, min(t, 5))
                        pt = psum_pool.tile([128, nt_out], mybir.dt.float32)
                        k = 0
                        for kh in range(3):
                            for kw in range(3):
                                xs, xb = xsrc, xbase
                                off = (h0 + kh) * W + kw - xb
                                # strided moving AP skips the 2 junk cols per
                                # row: [128, 2 chunks (stride 1B), rows
                                # (stride 112B), 54 cols (stride 2B)] — base
                                # byte 2*off is always even.
                                if mode == "fp8dr":
                                    rhs = xs[:, off : off + nt_in, :].rearrange(
                                        "p (r q) c -> p c r q", q=W
                                    )[:, :, :, 0:OW]
                                    nc.tensor.matmul(
                                        pt,
                                        w_sel(oc, k),
                                        rhs,
                                        start=(k == 0),
                                        stop=(k == 8),
                                        perf_mode=mybir.MatmulPerfMode.DoubleRow,
                                    )
                                else:
                                    for c in range(2):
                                        rhs = xs[:, off : off + nt_in, c].rearrange(
                                            "p (r q) -> p r q", q=W
                                        )[:, :, 0:OW]
                                        nc.tensor.matmul(
                                            pt,
                                            w_sel(oc, k)[:, c],
                                            rhs,
                                            start=(k == 0 and c == 0),
                                            stop=(k == 8 and c == 1),
                                        )
                                k += 1
                        nc.vector.tensor_copy(
                            out=ot[:, ob : ob + nt_out], in_=pt
                        )
                        if last_block:
                            # fine-grained stores on the final block: pairs
                            # early, singles at the end so the final store
                            # (and its HBM write receipt, which the exit
                            # barrier waits on) covers only 4 rows. The
                            # 5-row and 4-row pieces go on different rings.
                            if t in (1, 3):
                                nc.sync.dma_start(
                                    out=out_d[n, oc * 128 : (oc + 1) * 128,
                                              h0 - ROWS_PER_TILE : h0 + rows, :],
                                    in_=ot[:, ob - ROWS_PER_TILE * OW : ob + nt_out]
                                    .rearrange("p (h w) -> p h w", w=OW),
                                )
                            elif t >= 4:
                                ring = nc.scalar if t == len(tiles) - 1 else nc.sync
                                ring.dma_start(
                                    out=out_d[n, oc * 128 : (oc + 1) * 128,
                                              h0 : h0 + rows, :],
                                    in_=ot[:, ob : ob + nt_out].rearrange(
                                        "p (h w) -> p h w", w=OW
                                    ),
                                )
                    if not last_block:
                        nc.sync.dma_start(
                            out=out_d[n, oc * 128 : (oc + 1) * 128, :, :],
                            in_=ot.rearrange("p (h w) -> p h w", w=OW),
                        )
    nc.compile()
    return nc


def get_program(mode="fp8dr"):
    if mode not in _PROGRAM_CACHE:
        _PROGRAM_CACHE[mode] = _build_program(mode)
    return _PROGRAM_CACHE[mode]


def _np_dtype(mode):
    return ml_dtypes.float8_e4m3 if mode == "fp8dr" else ml_dtypes.bfloat16


def prep_weight(weight, mode="fp8dr"):
    """weight [256, 256, 3, 3] OIHW fp32 -> w_sb [128 ki, 2 oc, 9 tap, 2 c, 128 m]."""
    wq = weight.astype(np.int32).astype(np.float32)
    wq = wq.reshape(2, 128, 2, 128, 3, 3)  # [oc, m, c, ki, kh, kw]
    w_sb = np.ascontiguousarray(wq.transpose(3, 0, 4, 5, 2, 1))  # [ki, oc, kh, kw, c, m]
    w_sb = w_sb.reshape(128, 2, 9, 2, 128)
    return w_sb.astype(_np_dtype(mode))


def prep_x_core(x_core, mode="fp8dr"):
    """x_core [IMGS, 256, 56, 56] int32 -> x_sb [128 ki, IMGS, PIXP, 2 c]."""
    xq = np.clip(x_core.astype(np.int32), 0, 7).astype(np.float32)
    xq = xq.reshape(IMGS, 2, 128, PIX)  # [n, c, ki, pix]
    x_sb = np.zeros((128, IMGS, PIXP, 2), np.float32)
    x_sb[:, :, :PIX, :] = xq.transpose(2, 0, 3, 1)
    return x_sb.astype(_np_dtype(mode))


def make_in_maps(x, weight, mode="fp8dr"):
    w_sb = prep_weight(weight, mode)
    return [
        {"x_sb": prep_x_core(x[c * IMGS : (c + 1) * IMGS], mode), "w_sb": w_sb}
        for c in range(N_CORES)
    ]


def kernel(x, weight):
    import time

    from concourse.bass_utils import run_bass_kernel_spmd

    mode = "fp8dr"
    nc = get_program(mode)
    in_maps = make_in_maps(np.asarray(x), np.asarray(weight), mode)
    last_err = None
    for attempt in range(3):
        try:
            res = run_bass_kernel_spmd(nc, in_maps, list(range(N_CORES)))
            break
        except Exception as e:  # transient NRT_EXEC_UNIT_UNRECOVERABLE flakes
            last_err = e
            time.sleep(2.0)
    else:
        raise last_err
    return np.concatenate(
        [res.results[c]["out"] for c in range(N_CORES)], axis=0
    ).astype(np.float32)


# revision 28
# speedup vs baseline: 1.2046x; 1.0091x over previous
"""Trainium2 Bass kernel for DirectConv2D (3x3 VALID, NCHW/OIHW).

Problem: x [32, 256, 56, 56] int32 (values 0..7 after clip),
         weight [256, 256, 3, 3] fp32 (small non-negative ints 0..6)
         -> out [32, 256, 54, 54] fp32.

Strategy:
 - Data-parallel across 8 NeuronCores: 4 images per core, weight replicated.
 - Conv decomposed into 9 shifted matmuls (one per kernel tap) accumulated
   in PSUM; contraction over the 256 input channels.
 - Inputs are tiny non-negative integers, so fp8-e4m3 matmuls are exact
   (products <= 42, fp32 PSUM accumulation). DoubleRow perf mode contracts
   all 256 input channels (2 x 128-partition k-tiles) per matmul.
 - Activations live in SBUF with the two 128-channel chunks INTERLEAVED
   at adjacent bytes: [128 part, pix 3140, chunk 2] (56*56=3136 pixels +
   4 pad so every tile can read a full 504-wide window). The DoubleRow
   moving AP then steps 2 bytes per pixel with the chunk pair contiguous,
   so every tap's base byte is EVEN — avoiding the +7.5ns/matmul odd-base
   issue penalty kw=1 taps would otherwise pay — and each image loads in
   a single DMA. Output computed in tiles of 9 rows x 56 cols = 504 <=
   512 (one PSUM bank); only the 54 valid cols per row are stored.
 - Output staged/stored as fp16 (exact ints up to ~27k round to <=8 abs,
   ~3e-4 rel — far within tolerance) halving store traffic; host upcasts.
 - Head: tiny warm-up memsets on DVE (free + boots early), critical input
   DMAs issued first across the sync AND scalar rings in parallel, and a
   fine-grained junk-MM bridge (3x486 + 18x243 cols) keeps the PE busy
   until the first real data lands (10.5-12.6us under 8-core HBM
   contention) — any PE idle gap resets the HAM un-throttle window
   (~2-4us of half-rate), while bridge overrun costs only ~0.2us steps.
 - Input loads are HBM-bandwidth-bound across the 8 cores, so total load
   bytes are kept minimal (no odd-shifted x duplicates), and the oc0
   weight block loads as ONE tile so the first accumulation group waits
   on a single DMA completion (a taps-0-2/3-8 split stalled mid-group on
   unlucky cores).
 - Tail: last block tiled 9,9,9,9,9,5,4 rows so the final PSUM-evacuate
   -> store -> HBM-write-receipt chain (which the exit barrier serializes
   on) covers only 4 rows, with the 5/4-row stores on different rings.
"""

import sys

sys.path.insert(0, "/opt/trn_rl_repo")

import ml_dtypes
import numpy as np

N_CORES = 8
IMGS = 4  # images per core
H = W = 56
OH = OW = 54
PIX = H * W  # 3136
PIXP = PIX + 4  # padded so kh=2,kw=2 window of width 504 stays in-bounds
ROWS_PER_TILE = 9
N_TILE = ROWS_PER_TILE * W  # 504 (<= 512 fp32 PSUM bank)
N_ROWTILES = OH // ROWS_PER_TILE  # 6

_PROGRAM_CACHE = {}


def _build_program(mode="fp8dr"):
    import concourse.bacc as bacc
    import concourse.mybir as mybir
    import concourse.tile as tile

    nc = bacc.Bacc(
        "TRN2",
        target_bir_lowering=False,
        debug=False,
        enable_asserts=False,
        num_devices=N_CORES,
    )
    dt8 = mybir.dt.float8e4
    dtb = mybir.dt.bfloat16
    dt_in = dt8 if mode == "fp8dr" else dtb
    dt_out = mybir.dt.float16

    x_d = nc.dram_tensor("x_sb", [128, IMGS, PIXP, 2], dt_in, kind="ExternalInput").ap()
    w_d = nc.dram_tensor("w_sb", [128, 2, 9, 2, 128], dt_in, kind="ExternalInput").ap()
    out_d = nc.dram_tensor(
        "out", [IMGS, 256, OH, OW], dt_out, kind="ExternalOutput"
    ).ap()

    NT486 = ROWS_PER_TILE * OW  # 486 output pixels per row tile
    X0A_END = 1232  # image-0 leading tile: rows 0..21 (covers row tiles 0,1)
    X0M_BASE, X0M_END = 1008, 2140  # image-0 middle tile (row tiles 2,3)
    X0Z_BASE = 2016  # image-0 trailing tile (row tiles 4,5)

    with tile.TileContext(nc) as tc:
        with (
            tc.tile_pool(name="const", bufs=1) as const_pool,
            tc.tile_pool(name="psum", bufs=8, space="PSUM") as psum_pool,
            tc.tile_pool(name="outs", bufs=3) as out_pool,
        ):
            # PE warm-up on scratch: a handful of matmuls bridging the gap
            # between engine start and the first real input data landing, so
            # HAM un-throttling begins ASAP. Results are never read (next
            # user of the PSUM slot starts with start=True). Memsets go on
            # DVE (idle until the first PSUM evacuation ~15us in) so the
            # warm-up's only dependency clears within ~100ns of engine start.
            w_warm = const_pool.tile([128, 2, 128], dt_in)
            x_warm = const_pool.tile([128, 2, 544], dt_in)
            if mode != "fp8dr":
                nc.vector.memset(w_warm, 0.0)
                nc.vector.memset(x_warm, 0.0)
            else:
                # tiles must have a writer to be allocated; a 2-byte memset
                # is enough and keeps the warm-up dependency nearly free
                nc.vector.memset(w_warm[:, 0, 0:2], 0.0)
                nc.vector.memset(x_warm[:, 0, 0:2], 0.0)
            # Under 8-core HBM contention the first input chunks land ~10us
            # in; junk MMs bridge the whole window — any PE idle gap before
            # the HAM SHORT window completes resets the un-throttle clock
            # (costs ~2-4us of half-rate), so over-bridging is the safe side.
            # Junk bridge: 3 full-width MMs (pipeline fill) then short ones,
            # so however late the first real data lands (10.5-12.6us spread
            # under 8-core HBM contention), the PE stays continuously busy
            # (a gap resets the HAM un-throttle window, ~2-4us of half-rate)
            # while the overrun past data-ready is at most ~0.2us.
            pt_warm = psum_pool.tile([128, NT486], mybir.dt.float32, tag="pt")
            warm_ns = [486] * 3 + [243] * 18
            for i, nw in enumerate(warm_ns):
                rhs_w = x_warm[:, :, 0:nw].rearrange(
                    "p c (r q) -> p c r q", q=81
                )
                if mode == "fp8dr":
                    nc.tensor.matmul(
                        pt_warm[:, 0:nw], w_warm, rhs_w,
                        start=(i == 0), stop=(i == len(warm_ns) - 1),
                        perf_mode=mybir.MatmulPerfMode.DoubleRow,
                    )
                else:
                    nc.tensor.matmul(
                        pt_warm[:, 0:nw], w_warm[:, 0], rhs_w[:, 0],
                        start=(i == 0), stop=(i == len(warm_ns) - 1),
                    )

            # Weights split into three tiles so dependency tracking (which is
            # per-tile) lets the first accumulation group start as soon as the
            # small taps-0..2 chunk lands, instead of the whole 4.6KB/part.
            wt0 = const_pool.tile([128, 9, 2, 128], dt_in)  # oc0 all taps
            wt1 = const_pool.tile([128, 9, 2, 128], dt_in)  # oc1 all taps
            # Per-image x tiles so matmul deps only cover the image they
            # read (dependency tracking is per-tile). Chunk-interleaved:
            # [128, pix, 2].
            xt0a = const_pool.tile([128, X0A_END, 2], dt_in)
            xt0m = const_pool.tile([128, X0M_END - X0M_BASE, 2], dt_in)
            xt0z = const_pool.tile([128, PIXP - X0Z_BASE, 2], dt_in)
            xts = [None] + [
                const_pool.tile([128, PIXP, 2], dt_in, name=f"xt{n}", tag=f"xt{n}")
                for n in (1, 2, 3)
            ]
            # Pixel-shifted duplicates of images 1-3 (pixel p at slot p+1):
            # matmuls whose moving-AP base is not 4B-aligned run +7.5ns
            # (measured: exactly the kw=1 taps, base 2*off with off odd).
            # Reading the shifted copy makes the kw=1 base = 2*(off+1), a
            # multiple of 4: 36 groups x 3 taps x 7.5ns ~ 0.8us.
            xto = [None] + [
                const_pool.tile(
                    [128, PIXP + 1, 2], dt_in, name=f"xo{n}", tag=f"xo{n}"
                )
                for n in (1, 2, 3)
            ]
            # dma_start issue costs ~600ns serialized per sequencer, so the
            # first-needed bytes go at slot 0 of BOTH hw rings in parallel
            # (image-0 lead on sync, its first weight taps on scalar);
            # everything later is ordered by first-use time.
            nc.sync.dma_start(out=xt0a[:, 0:620], in_=x_d[:, 0, 0:620])
            nc.sync.dma_start(out=xt0a[:, 620:], in_=x_d[:, 0, 620:X0A_END])
            nc.sync.dma_start(out=xt0m, in_=x_d[:, 0, X0M_BASE:X0M_END])
            nc.sync.dma_start(out=wt1, in_=w_d[:, 1])
            nc.sync.dma_start(out=xts[1], in_=x_d[:, 1])
            nc.sync.dma_start(out=xts[3], in_=x_d[:, 3])
            nc.scalar.dma_start(out=wt0, in_=w_d[:, 0])
            nc.scalar.dma_start(out=xt0z, in_=x_d[:, 0, X0Z_BASE:])
            nc.scalar.dma_start(out=xts[2], in_=x_d[:, 2])
            # The duplicate loads are pure overhead for the HBM-bound head
            # window, so gate each on the LAST even load having landed (tiny
            # DVE copy reading xts[3]'s tail into the dup's slot range, so
            # the tracker orders copy -> DMA): they stream ~18-26us, well
            # before first use at ~33us.
            for n in (1, 2, 3):
                nc.vector.tensor_copy(
                    out=xto[n][:, 1:2, :], in_=xts[3][:, PIXP - 1 : PIXP, :]
                )
            nc.sync.dma_start(out=xto[1][:, 1 : 1 + PIXP], in_=x_d[:, 1])
            nc.scalar.dma_start(out=xto[2][:, 1 : 1 + PIXP], in_=x_d[:, 2])
            nc.sync.dma_start(out=xto[3][:, 1 : 1 + PIXP], in_=x_d[:, 3])

            def x_src(n, t):
                """(x tile, pixel base) holding rows needed by row tile t."""
                if n == 0:
                    if t < 2:
                        return xt0a, 0
                    if t < 4:
                        return xt0m, X0M_BASE
                    return xt0z, X0Z_BASE
                return xts[n], 0

            def w_sel(oc, k):
                """Stationary weight AP [128, 2, 128] for (oc, tap k)."""
                return (wt0 if oc == 0 else wt1)[:, k]

            # Last block gets a small trailing row-tile so the final
            # PSUM-evacuate -> store -> HBM-write-receipt chain (which the
            # exit barrier serializes on) covers only 4 rows.
            TILES = [(t * ROWS_PER_TILE, ROWS_PER_TILE) for t in range(N_ROWTILES)]
            TILES_LAST = TILES[:5] + [(45, 5), (50, 4)]

            for n in range(IMGS):
                for oc in range(2):
                    last_block = n == IMGS - 1 and oc == 1
                    tiles = TILES_LAST if last_block else TILES
                    # staging for a full (n, oc) output block: dense 54x54
                    # rows so stores move 5.8KB-contiguous lines/partition.
                    ot = out_pool.tile([128, OH * OW], dt_out)
                    for t, (h0, rows) in enumerate(tiles):
                        nt_in = rows * W
                        nt_out = rows * OW
                        ob = h0 * OW  # fp16 staging offset of this tile
                        xsrc, xbase = x_src(n, min(t, 5))
                        pt = psum_pool.tile([128, nt_out], mybir.dt.float32)
                        k = 0
                        for kh in range(3):
                            for kw in range(3):
                                # kw=1 taps on images 1-3 read the shifted
                                # copy so their base byte is 4B-aligned
                                if n >= 1 and kw == 1:
                                    xs, xb = xto[n], -1
                                else:
                                    xs, xb = xsrc, xbase
                                off = (h0 + kh) * W + kw - xb
                                # strided moving AP skips the 2 junk cols per
                                # row: [128, 2 chunks (stride 1B), rows
                                # (stride 112B), 54 cols (stride 2B)] — base
                                # byte 2*off is always even.
                                if mode == "fp8dr":
                                    rhs = xs[:, off : off + nt_in, :].rearrange(
                                        "p (r q) c -> p c r q", q=W
                                    )[:, :, :, 0:OW]
                                    nc.tensor.matmul(
                                        pt,
                                        w_sel(oc, k),
                                        rhs,
                                        start=(k == 0),
                                        stop=(k == 8),
                                        perf_mode=mybir.MatmulPerfMode.DoubleRow,
                                    )
                                else:
                                    for c in range(2):
                                        rhs = xs[:, off : off + nt_in, c].rearrange(
                                            "p (r q) -> p r q", q=W
                                        )[:, :, 0:OW]
                                        nc.tensor.matmul(
                                            pt,
                                            w_sel(oc, k)[:, c],
                                            rhs,
                                            start=(k == 0 and c == 0),
                                            stop=(k == 8 and c == 1),
                                        )
                                k += 1
                        nc.vector.tensor_copy(
                            out=ot[:, ob : ob + nt_out], in_=pt
                        )
                        if last_block:
                            # fine-grained stores on the final block: pairs
                            # early, singles at the end so the final store
                            # (and its HBM write receipt, which the exit
                            # barrier waits on) covers only 4 rows. The
                            # 5-row and 4-row pieces go on different rings.
                            if t in (1, 3):
                                nc.sync.dma_start(
                                    out=out_d[n, oc * 128 : (oc + 1) * 128,
                                              h0 - ROWS_PER_TILE : h0 + rows, :],
                                    in_=ot[:, ob - ROWS_PER_TILE * OW : ob + nt_out]
                                    .rearrange("p (h w) -> p h w", w=OW),
                                )
                            elif t >= 4:
                                ring = nc.scalar if t == len(tiles) - 1 else nc.sync
                                ring.dma_start(
                                    out=out_d[n, oc * 128 : (oc + 1) * 128,
                                              h0 : h0 + rows, :],
                                    in_=ot[:, ob : ob + nt_out].rearrange(
                                        "p (h w) -> p h w", w=OW
                                    ),
                                )
                    if not last_block:
                        nc.sync.dma_start(
                            out=out_d[n, oc * 128 : (oc + 1) * 128, :, :],
                            in_=ot.rearrange("p (h w) -> p h w", w=OW),
                        )
    nc.compile()
    return nc


def get_program(mode="fp8dr"):
    if mode not in _PROGRAM_CACHE:
        _PROGRAM_CACHE[mode] = _build_program(mode)
    return _PROGRAM_CACHE[mode]


def _np_dtype(mode):
    return ml_dtypes.float8_e4m3 if mode == "fp8dr" else ml_dtypes.bfloat16


def prep_weight(weight, mode="fp8dr"):
    """weight [256, 256, 3, 3] OIHW fp32 -> w_sb [128 ki, 2 oc, 9 tap, 2 c, 128 m]."""
    wq = weight.astype(np.int32).astype(np.float32)
    wq = wq.reshape(2, 128, 2, 128, 3, 3)  # [oc, m, c, ki, kh, kw]
    w_sb = np.ascontiguousarray(wq.transpose(3, 0, 4, 5, 2, 1))  # [ki, oc, kh, kw, c, m]
    w_sb = w_sb.reshape(128, 2, 9, 2, 128)
    return w_sb.astype(_np_dtype(mode))


def prep_x_core(x_core, mode="fp8dr"):
    """x_core [IMGS, 256, 56, 56] int32 -> x_sb [128 ki, IMGS, PIXP, 2 c]."""
    xq = np.clip(x_core.astype(np.int32), 0, 7).astype(np.float32)
    xq = xq.reshape(IMGS, 2, 128, PIX)  # [n, c, ki, pix]
    x_sb = np.zeros((128, IMGS, PIXP, 2), np.float32)
    x_sb[:, :, :PIX, :] = xq.transpose(2, 0, 3, 1)
    return x_sb.astype(_np_dtype(mode))


def make_in_maps(x, weight, mode="fp8dr"):
    w_sb = prep_weight(weight, mode)
    return [
        {"x_sb": prep_x_core(x[c * IMGS : (c + 1) * IMGS], mode), "w_sb": w_sb}
        for c in range(N_CORES)
    ]


def kernel(x, weight):
    import time

    from concourse.bass_utils import run_bass_kernel_spmd

    mode = "fp8dr"
    nc = get_program(mode)
    in_maps = make_in_maps(np.asarray(x), np.asarray(weight), mode)
    last_err = None
    for attempt in range(3):
        try:
            res = run_bass_kernel_spmd(nc, in_maps, list(range(N_CORES)))
            break
        except Exception as e:  # transient NRT_EXEC_UNIT_UNRECOVERABLE flakes
            last_err = e
            time.sleep(2.0)
    else:
        raise last_err
    return np.concatenate(
        [res.results[c]["out"] for c in range(N_CORES)], axis=0
    ).astype(np.float32)


# revision 29
# speedup vs baseline: 1.2068x; 1.0019x over previous
"""Trainium2 Bass kernel for DirectConv2D (3x3 VALID, NCHW/OIHW).

Problem: x [32, 256, 56, 56] int32 (values 0..7 after clip),
         weight [256, 256, 3, 3] fp32 (small non-negative ints 0..6)
         -> out [32, 256, 54, 54] fp32.

Strategy:
 - Data-parallel across 8 NeuronCores: 4 images per core, weight replicated.
 - Conv decomposed into 9 shifted matmuls (one per kernel tap) accumulated
   in PSUM; contraction over the 256 input channels.
 - Inputs are tiny non-negative integers, so fp8-e4m3 matmuls are exact
   (products <= 42, fp32 PSUM accumulation). DoubleRow perf mode contracts
   all 256 input channels (2 x 128-partition k-tiles) per matmul.
 - Activations live in SBUF with the two 128-channel chunks INTERLEAVED
   at adjacent bytes: [128 part, pix 3140, chunk 2] (56*56=3136 pixels +
   4 pad so every tile can read a full 504-wide window). The DoubleRow
   moving AP then steps 2 bytes per pixel with the chunk pair contiguous,
   so every tap's base byte is EVEN — avoiding the +7.5ns/matmul odd-base
   issue penalty kw=1 taps would otherwise pay — and each image loads in
   a single DMA. Output computed in tiles of 9 rows x 56 cols = 504 <=
   512 (one PSUM bank); only the 54 valid cols per row are stored.
 - Output staged/stored as fp16 (exact ints up to ~27k round to <=8 abs,
   ~3e-4 rel — far within tolerance) halving store traffic; host upcasts.
 - Head: tiny warm-up memsets on DVE (free + boots early), critical input
   DMAs issued first across the sync AND scalar rings in parallel, and a
   fine-grained junk-MM bridge (3x486 + 18x243 cols) keeps the PE busy
   until the first real data lands (10.5-12.6us under 8-core HBM
   contention) — any PE idle gap resets the HAM un-throttle window
   (~2-4us of half-rate), while bridge overrun costs only ~0.2us steps.
 - Input loads are HBM-bandwidth-bound across the 8 cores, so total load
   bytes are kept minimal (no odd-shifted x duplicates), and the oc0
   weight block loads as ONE tile so the first accumulation group waits
   on a single DMA completion (a taps-0-2/3-8 split stalled mid-group on
   unlucky cores).
 - Tail: last block tiled 9,9,9,9,9,5,4 rows so the final PSUM-evacuate
   -> store -> HBM-write-receipt chain (which the exit barrier serializes
   on) covers only 4 rows, with the 5/4-row stores on different rings.
"""

import sys

sys.path.insert(0, "/opt/trn_rl_repo")

import ml_dtypes
import numpy as np

N_CORES = 8
IMGS = 4  # images per core
H = W = 56
OH = OW = 54
PIX = H * W  # 3136
PIXP = PIX + 4  # padded so kh=2,kw=2 window of width 504 stays in-bounds
ROWS_PER_TILE = 9
N_TILE = ROWS_PER_TILE * W  # 504 (<= 512 fp32 PSUM bank)
N_ROWTILES = OH // ROWS_PER_TILE  # 6

_PROGRAM_CACHE = {}


def _build_program(mode="fp8dr"):
    import concourse.bacc as bacc
    import concourse.mybir as mybir
    import concourse.tile as tile

    nc = bacc.Bacc(
        "TRN2",
        target_bir_lowering=False,
        debug=False,
        enable_asserts=False,
        num_devices=N_CORES,
    )
    dt8 = mybir.dt.float8e4
    dtb = mybir.dt.bfloat16
    dt_in = dt8 if mode == "fp8dr" else dtb
    dt_out = mybir.dt.float16

    x_d = nc.dram_tensor("x_sb", [128, IMGS, PIXP, 2], dt_in, kind="ExternalInput").ap()
    w_d = nc.dram_tensor("w_sb", [128, 2, 9, 2, 128], dt_in, kind="ExternalInput").ap()
    out_d = nc.dram_tensor(
        "out", [IMGS, 256, OH, OW], dt_out, kind="ExternalOutput"
    ).ap()

    NT486 = ROWS_PER_TILE * OW  # 486 output pixels per row tile
    X0A_END = 1232  # image-0 leading tile: rows 0..21 (covers row tiles 0,1)
    X0M_BASE, X0M_END = 1008, 2140  # image-0 middle tile (row tiles 2,3)
    X0Z_BASE = 2016  # image-0 trailing tile (row tiles 4,5)

    with tile.TileContext(nc) as tc:
        with (
            tc.tile_pool(name="const", bufs=1) as const_pool,
            tc.tile_pool(name="psum", bufs=8, space="PSUM") as psum_pool,
            tc.tile_pool(name="outs", bufs=3) as out_pool,
        ):
            # PE warm-up on scratch: a handful of matmuls bridging the gap
            # between engine start and the first real input data landing, so
            # HAM un-throttling begins ASAP. Results are never read (next
            # user of the PSUM slot starts with start=True). Memsets go on
            # DVE (idle until the first PSUM evacuation ~15us in) so the
            # warm-up's only dependency clears within ~100ns of engine start.
            w_warm = const_pool.tile([128, 2, 128], dt_in)
            x_warm = const_pool.tile([128, 2, 544], dt_in)
            if mode != "fp8dr":
                nc.vector.memset(w_warm, 0.0)
                nc.vector.memset(x_warm, 0.0)
            else:
                # tiles must have a writer to be allocated; a 2-byte memset
                # is enough and keeps the warm-up dependency nearly free
                nc.vector.memset(w_warm[:, 0, 0:2], 0.0)
                nc.vector.memset(x_warm[:, 0, 0:2], 0.0)
            # Under 8-core HBM contention the first input chunks land ~10us
            # in; junk MMs bridge the whole window — any PE idle gap before
            # the HAM SHORT window completes resets the un-throttle clock
            # (costs ~2-4us of half-rate), so over-bridging is the safe side.
            # Junk bridge: 3 full-width MMs (pipeline fill) then short ones,
            # so however late the first real data lands (10.5-12.6us spread
            # under 8-core HBM contention), the PE stays continuously busy
            # (a gap resets the HAM un-throttle window, ~2-4us of half-rate)
            # while the overrun past data-ready is at most ~0.2us.
            pt_warm = psum_pool.tile([128, NT486], mybir.dt.float32, tag="pt")
            # Sized so the slowest-landing cores see a sub-0.5us gap at
            # worst (measured: gaps that small leave only 1-2 cold MMs; the
            # HAM reset needs ~1.5us+ of idle), while typical cores save
            # the overrun.
            warm_ns = [486] * 3 + [243] * 16
            for i, nw in enumerate(warm_ns):
                rhs_w = x_warm[:, :, 0:nw].rearrange(
                    "p c (r q) -> p c r q", q=81
                )
                if mode == "fp8dr":
                    nc.tensor.matmul(
                        pt_warm[:, 0:nw], w_warm, rhs_w,
                        start=(i == 0), stop=(i == len(warm_ns) - 1),
                        perf_mode=mybir.MatmulPerfMode.DoubleRow,
                    )
                else:
                    nc.tensor.matmul(
                        pt_warm[:, 0:nw], w_warm[:, 0], rhs_w[:, 0],
                        start=(i == 0), stop=(i == len(warm_ns) - 1),
                    )

            # Weights split into three tiles so dependency tracking (which is
            # per-tile) lets the first accumulation group start as soon as the
            # small taps-0..2 chunk lands, instead of the whole 4.6KB/part.
            wt0 = const_pool.tile([128, 9, 2, 128], dt_in)  # oc0 all taps
            wt1 = const_pool.tile([128, 9, 2, 128], dt_in)  # oc1 all taps
            # Per-image x tiles so matmul deps only cover the image they
            # read (dependency tracking is per-tile). Chunk-interleaved:
            # [128, pix, 2].
            xt0a = const_pool.tile([128, X0A_END, 2], dt_in)
            xt0m = const_pool.tile([128, X0M_END - X0M_BASE, 2], dt_in)
            xt0z = const_pool.tile([128, PIXP - X0Z_BASE, 2], dt_in)
            xts = [None] + [
                const_pool.tile([128, PIXP, 2], dt_in, name=f"xt{n}", tag=f"xt{n}")
                for n in (1, 2, 3)
            ]
            # Pixel-shifted duplicates of images 1-3 (pixel p at slot p+1):
            # matmuls whose moving-AP base is not 4B-aligned run +7.5ns
            # (measured: exactly the kw=1 taps, base 2*off with off odd).
            # Reading the shifted copy makes the kw=1 base = 2*(off+1), a
            # multiple of 4: 36 groups x 3 taps x 7.5ns ~ 0.8us.
            xto = [None] + [
                const_pool.tile(
                    [128, PIXP + 1, 2], dt_in, name=f"xo{n}", tag=f"xo{n}"
                )
                for n in (1, 2, 3)
            ]
            # dma_start issue costs ~600ns serialized per sequencer, so the
            # first-needed bytes go at slot 0 of BOTH hw rings in parallel
            # (image-0 lead on sync, its first weight taps on scalar);
            # everything later is ordered by first-use time.
            nc.sync.dma_start(out=xt0a[:, 0:620], in_=x_d[:, 0, 0:620])
            nc.sync.dma_start(out=xt0a[:, 620:], in_=x_d[:, 0, 620:X0A_END])
            nc.sync.dma_start(out=xt0m, in_=x_d[:, 0, X0M_BASE:X0M_END])
            nc.sync.dma_start(out=wt1, in_=w_d[:, 1])
            nc.sync.dma_start(out=xts[1], in_=x_d[:, 1])
            nc.sync.dma_start(out=xts[3], in_=x_d[:, 3])
            nc.scalar.dma_start(out=wt0, in_=w_d[:, 0])
            nc.scalar.dma_start(out=xt0z, in_=x_d[:, 0, X0Z_BASE:])
            nc.scalar.dma_start(out=xts[2], in_=x_d[:, 2])
            # The duplicate loads are pure overhead for the HBM-bound head
            # window, so gate each on the LAST even load having landed (tiny
            # DVE copy reading xts[3]'s tail into the dup's slot range, so
            # the tracker orders copy -> DMA): they stream ~18-26us, well
            # before first use at ~33us.
            for n in (1, 2, 3):
                nc.vector.tensor_copy(
                    out=xto[n][:, 1:2, :], in_=xts[3][:, PIXP - 1 : PIXP, :]
                )
            nc.sync.dma_start(out=xto[1][:, 1 : 1 + PIXP], in_=x_d[:, 1])
            nc.scalar.dma_start(out=xto[2][:, 1 : 1 + PIXP], in_=x_d[:, 2])
            nc.sync.dma_start(out=xto[3][:, 1 : 1 + PIXP], in_=x_d[:, 3])

            def x_src(n, t):
                """(x tile, pixel base) holding rows needed by row tile t."""
                if n == 0:
                    if t < 2:
                        return xt0a, 0
                    if t < 4:
                        return xt0m, X0M_BASE
                    return xt0z, X0Z_BASE
                return xts[n], 0

            def w_sel(oc, k):
                """Stationary weight AP [128, 2, 128] for (oc, tap k)."""
                return (wt0 if oc == 0 else wt1)[:, k]

            # Last block gets a small trailing row-tile so the final
            # PSUM-evacuate -> store -> HBM-write-receipt chain (which the
            # exit barrier serializes on) covers only 4 rows.
            TILES = [(t * ROWS_PER_TILE, ROWS_PER_TILE) for t in range(N_ROWTILES)]
            TILES_LAST = TILES[:5] + [(45, 5), (50, 4)]

            for n in range(IMGS):
                for oc in range(2):
                    last_block = n == IMGS - 1 and oc == 1
                    tiles = TILES_LAST if last_block else TILES
                    # staging for a full (n, oc) output block: dense 54x54
                    # rows so stores move 5.8KB-contiguous lines/partition.
                    ot = out_pool.tile([128, OH * OW], dt_out)
                    for t, (h0, rows) in enumerate(tiles):
                        nt_in = rows * W
                        nt_out = rows * OW
                        ob = h0 * OW  # fp16 staging offset of this tile
                        xsrc, xbase = x_src(n, min(t, 5))
                        pt = psum_pool.tile([128, nt_out], mybir.dt.float32)
                        k = 0
                        for kh in range(3):
                            for kw in range(3):
                                # kw=1 taps on images 1-3 read the shifted
                                # copy so their base byte is 4B-aligned
                                if n >= 1 and kw == 1:
                                    xs, xb = xto[n], -1
                                else:
                                    xs, xb = xsrc, xbase
                                off = (h0 + kh) * W + kw - xb
                                # strided moving AP skips the 2 junk cols per
                                # row: [128, 2 chunks (stride 1B), rows
                                # (stride 112B), 54 cols (stride 2B)] — base
                                # byte 2*off is always even.
                                if mode == "fp8dr":
                                    rhs = xs[:, off : off + nt_in, :].rearrange(
                                        "p (r q) c -> p c r q", q=W
                                    )[:, :, :, 0:OW]
                                    nc.tensor.matmul(
                                        pt,
                                        w_sel(oc, k),
                                        rhs,
                                        start=(k == 0),
                                        stop=(k == 8),
                                        perf_mode=mybir.MatmulPerfMode.DoubleRow,
                                    )
                                else:
                                    for c in range(2):
                                        rhs = xs[:, off : off + nt_in, c].rearrange(
                                            "p (r q) -> p r q", q=W
                                        )[:, :, 0:OW]
                                        nc.tensor.matmul(
                                            pt,
                                            w_sel(oc, k)[:, c],
                                            rhs,
                                            start=(k == 0 and c == 0),
                                            stop=(k == 8 and c == 1),
                                        )
                                k += 1
                        nc.vector.tensor_copy(
                            out=ot[:, ob : ob + nt_out], in_=pt
                        )
                        if last_block:
                            # fine-grained stores on the final block: pairs
                            # early, singles at the end so the final store
                            # (and its HBM write receipt, which the exit
                            # barrier waits on) covers only 4 rows. The
                            # 5-row and 4-row pieces go on different rings.
                            if t in (1, 3):
                                nc.sync.dma_start(
                                    out=out_d[n, oc * 128 : (oc + 1) * 128,
                                              h0 - ROWS_PER_TILE : h0 + rows, :],
                                    in_=ot[:, ob - ROWS_PER_TILE * OW : ob + nt_out]
                                    .rearrange("p (h w) -> p h w", w=OW),
                                )
                            elif t >= 4:
                                ring = nc.scalar if t == len(tiles) - 1 else nc.sync
                                ring.dma_start(
                                    out=out_d[n, oc * 128 : (oc + 1) * 128,
                                              h0 : h0 + rows, :],
                                    in_=ot[:, ob : ob + nt_out].rearrange(
                                        "p (h w) -> p h w", w=OW
                                    ),
                                )
                    if not last_block:
                        nc.sync.dma_start(
                            out=out_d[n, oc * 128 : (oc + 1) * 128, :, :],
                            in_=ot.rearrange("p (h w) -> p h w", w=OW),
                        )
    nc.compile()
    return nc


def get_program(mode="fp8dr"):
    if mode not in _PROGRAM_CACHE:
        _PROGRAM_CACHE[mode] = _build_program(mode)
    return _PROGRAM_CACHE[mode]


def _np_dtype(mode):
    return ml_dtypes.float8_e4m3 if mode == "fp8dr" else ml_dtypes.bfloat16


def prep_weight(weight, mode="fp8dr"):
    """weight [256, 256, 3, 3] OIHW fp32 -> w_sb [128 ki, 2 oc, 9 tap, 2 c, 128 m]."""
    wq = weight.astype(np.int32).astype(np.float32)
    wq = wq.reshape(2, 128, 2, 128, 3, 3)  # [oc, m, c, ki, kh, kw]
    w_sb = np.ascontiguousarray(wq.transpose(3, 0, 4, 5, 2, 1))  # [ki, oc, kh, kw, c, m]
    w_sb = w_sb.reshape(128, 2, 9, 2, 128)
    return w_sb.astype(_np_dtype(mode))


def prep_x_core(x_core, mode="fp8dr"):
    """x_core [IMGS, 256, 56, 56] int32 -> x_sb [128 ki, IMGS, PIXP, 2 c]."""
    xq = np.clip(x_core.astype(np.int32), 0, 7).astype(np.float32)
    xq = xq.reshape(IMGS, 2, 128, PIX)  # [n, c, ki, pix]
    x_sb = np.zeros((128, IMGS, PIXP, 2), np.float32)
    x_sb[:, :, :PIX, :] = xq.transpose(2, 0, 3, 1)
    return x_sb.astype(_np_dtype(mode))


def make_in_maps(x, weight, mode="fp8dr"):
    w_sb = prep_weight(weight, mode)
    return [
        {"x_sb": prep_x_core(x[c * IMGS : (c + 1) * IMGS], mode), "w_sb": w_sb}
        for c in range(N_CORES)
    ]


def kernel(x, weight):
    import time

    from concourse.bass_utils import run_bass_kernel_spmd

    mode = "fp8dr"
    nc = get_program(mode)
    in_maps = make_in_maps(np.asarray(x), np.asarray(weight), mode)
    last_err = None
    for attempt in range(3):
        try:
            res = run_bass_kernel_spmd(nc, in_maps, list(range(N_CORES)))
            break
        except Exception as e:  # transient NRT_EXEC_UNIT_UNRECOVERABLE flakes
            last_err = e
            time.sleep(2.0)
    else:
        raise last_err
    return np.concatenate(
        [res.results[c]["out"] for c in range(N_CORES)], axis=0
    ).astype(np.float32)


# revision 33
# speedup vs baseline: 1.2105x; 1.0030x over previous
"""Trainium2 Bass kernel for DirectConv2D (3x3 VALID, NCHW/OIHW).

Problem: x [32, 256, 56, 56] int32 (values 0..7 after clip),
         weight [256, 256, 3, 3] fp32 (small non-negative ints 0..6)
         -> out [32, 256, 54, 54] fp32.

Strategy:
 - Data-parallel across 8 NeuronCores: 4 images per core, weight replicated.
 - Conv decomposed into 9 shifted matmuls (one per kernel tap) accumulated
   in PSUM; contraction over the 256 input channels.
 - Inputs are tiny non-negative integers, so fp8-e4m3 matmuls are exact
   (products <= 42, fp32 PSUM accumulation). DoubleRow perf mode contracts
   all 256 input channels (2 x 128-partition k-tiles) per matmul.
 - Activations live in SBUF with the two 128-channel chunks INTERLEAVED
   at adjacent bytes: [128 part, pix 3140, chunk 2] (56*56=3136 pixels +
   4 pad so every tile can read a full 504-wide window), so each image
   loads in a single DMA and the DoubleRow moving AP reads the chunk
   pair contiguously. Matmuls whose moving-AP base is not 4B-aligned pay
   +7.5ns (exactly the kw=1 taps); images 1-3 dodge it by reading
   pixel-shifted duplicates. Output computed in tiles of 9 rows x 56
   cols = 504 <= 512 (one PSUM bank); only 54 valid cols/row stored.
 - Output staged/stored as fp16 (exact ints up to ~27k round to <=8 abs,
   ~3e-4 rel — far within tolerance) halving store traffic; host upcasts.
 - Head: tiny warm-up memsets on DVE (free + boots early), critical input
   DMAs issued first across the sync AND scalar rings in parallel, and a
   fine-grained junk-MM bridge (3x486 + 16x243 cols) keeps the PE busy
   until the first real data lands (10.5-12.6us under 8-core HBM
   contention) — any PE idle gap resets the HAM un-throttle window
   (~2-4us of half-rate), while bridge overrun costs only ~0.2us steps.
 - Input loads are HBM-bandwidth-bound across the 8 cores, so total load
   bytes are kept minimal (no odd-shifted x duplicates), and the oc0
   weight block loads as ONE tile so the first accumulation group waits
   on a single DMA completion (a taps-0-2/3-8 split stalled mid-group on
   unlucky cores).
 - Tail: last block tiled 9,9,9,9,9,5,4 rows so the final PSUM-evacuate
   -> store -> HBM-write-receipt chain (which the exit barrier serializes
   on) covers only 4 rows, with the 5/4-row stores on different rings.
"""

import sys

sys.path.insert(0, "/opt/trn_rl_repo")

import ml_dtypes
import numpy as np

N_CORES = 8
IMGS = 4  # images per core
H = W = 56
OH = OW = 54
PIX = H * W  # 3136
PIXP = PIX + 4  # padded so kh=2,kw=2 window of width 504 stays in-bounds
ROWS_PER_TILE = 9
N_TILE = ROWS_PER_TILE * W  # 504 (<= 512 fp32 PSUM bank)
N_ROWTILES = OH // ROWS_PER_TILE  # 6

_PROGRAM_CACHE = {}


def _build_program(mode="fp8dr"):
    import concourse.bacc as bacc
    import concourse.mybir as mybir
    import concourse.tile as tile

    nc = bacc.Bacc(
        "TRN2",
        target_bir_lowering=False,
        debug=False,
        enable_asserts=False,
        num_devices=N_CORES,
    )
    dt8 = mybir.dt.float8e4
    dtb = mybir.dt.bfloat16
    dt_in = dt8 if mode == "fp8dr" else dtb
    dt_out = mybir.dt.float16

    x_d = nc.dram_tensor("x_sb", [128, IMGS, PIXP, 2], dt_in, kind="ExternalInput").ap()
    w_d = nc.dram_tensor("w_sb", [128, 2, 9, 2, 128], dt_in, kind="ExternalInput").ap()
    out_d = nc.dram_tensor(
        "out", [IMGS, 256, OH, OW], dt_out, kind="ExternalOutput"
    ).ap()

    NT486 = ROWS_PER_TILE * OW  # 486 output pixels per row tile
    X0A_END = 1232  # image-0 leading tile: rows 0..21 (covers row tiles 0,1)
    X0M_BASE, X0M_END = 1008, 2140  # image-0 middle tile (row tiles 2,3)
    X0Z_BASE = 2016  # image-0 trailing tile (row tiles 4,5)

    with tile.TileContext(nc) as tc:
        with (
            tc.tile_pool(name="const", bufs=1) as const_pool,
            tc.tile_pool(name="psum", bufs=8, space="PSUM") as psum_pool,
            tc.tile_pool(name="outs", bufs=3) as out_pool,
        ):
            # PE warm-up on scratch: a handful of matmuls bridging the gap
            # between engine start and the first real input data landing, so
            # HAM un-throttling begins ASAP. Results are never read (next
            # user of the PSUM slot starts with start=True). Memsets go on
            # DVE (idle until the first PSUM evacuation ~15us in) so the
            # warm-up's only dependency clears within ~100ns of engine start.
            w_warm = const_pool.tile([128, 2, 128], dt_in)
            x_warm = const_pool.tile([128, 2, 544], dt_in)
            if mode != "fp8dr":
                nc.vector.memset(w_warm, 0.0)
                nc.vector.memset(x_warm, 0.0)
            else:
                # tiles must have a writer to be allocated; a 2-byte memset
                # is enough and keeps the warm-up dependency nearly free
                nc.vector.memset(w_warm[:, 0, 0:2], 0.0)
                nc.vector.memset(x_warm[:, 0, 0:2], 0.0)
            # Junk bridge: 3 full-width MMs (pipeline fill) then short ones
            # until the first real data lands (10.5-12.6us spread under
            # 8-core HBM contention). A PE idle gap >~1.5us resets the HAM
            # un-throttle window (~2-4us of half-rate), sub-0.5us gaps cost
            # only 1-2 cold MMs; junk spacing is LDWEIGHTS-bound (~205ns),
            # so 243 cols is already the useful granularity floor.
            pt_warm = psum_pool.tile([128, NT486], mybir.dt.float32, tag="pt")
            warm_ns = [486] * 3 + [243] * 16
            for i, nw in enumerate(warm_ns):
                rhs_w = x_warm[:, :, 0:nw].rearrange(
                    "p c (r q) -> p c r q", q=81
                )
                if mode == "fp8dr":
                    nc.tensor.matmul(
                        pt_warm[:, 0:nw], w_warm, rhs_w,
                        start=(i == 0), stop=(i == len(warm_ns) - 1),
                        perf_mode=mybir.MatmulPerfMode.DoubleRow,
                    )
                else:
                    nc.tensor.matmul(
                        pt_warm[:, 0:nw], w_warm[:, 0], rhs_w[:, 0],
                        start=(i == 0), stop=(i == len(warm_ns) - 1),
                    )

            # One tile (= one DMA completion) per output-channel half: the
            # first accumulation group then waits on a single sem — a finer
            # taps-0-2/3-8 split stalled mid-group on slow-landing cores.
            wt0 = const_pool.tile([128, 9, 2, 128], dt_in)  # oc0 all taps
            wt1 = const_pool.tile([128, 9, 2, 128], dt_in)  # oc1 all taps
            # Per-image x tiles so matmul deps only cover the image they
            # read (dependency tracking is per-tile). Chunk-interleaved:
            # [128, pix, 2].
            xt0a = const_pool.tile([128, X0A_END, 2], dt_in)
            xt0m = const_pool.tile([128, X0M_END - X0M_BASE, 2], dt_in)
            xt0z = const_pool.tile([128, PIXP - X0Z_BASE, 2], dt_in)
            xts = [None] + [
                const_pool.tile([128, PIXP, 2], dt_in, name=f"xt{n}", tag=f"xt{n}")
                for n in (1, 2, 3)
            ]
            # Pixel-shifted duplicates of images 1-3 (pixel p at slot p+1):
            # matmuls whose moving-AP base is not 4B-aligned run +7.5ns
            # (measured: exactly the kw=1 taps, base 2*off with off odd).
            # Reading the shifted copy makes the kw=1 base = 2*(off+1), a
            # multiple of 4: 36 groups x 3 taps x 7.5ns ~ 0.8us.
            xto = [None] + [
                const_pool.tile(
                    [128, PIXP + 1, 2], dt_in, name=f"xo{n}", tag=f"xo{n}"
                )
                for n in (1, 2, 3)
            ]
            # dma_start issue costs ~600ns serialized per sequencer, so the
            # first-needed bytes go at slot 0 of BOTH hw rings in parallel
            # (image-0 lead on sync, its first weight taps on scalar);
            # everything later is ordered by first-use time.
            nc.sync.dma_start(out=xt0a[:, 0:620], in_=x_d[:, 0, 0:620])
            nc.sync.dma_start(out=xt0a[:, 620:], in_=x_d[:, 0, 620:X0A_END])
            nc.sync.dma_start(out=xt0m, in_=x_d[:, 0, X0M_BASE:X0M_END])
            nc.sync.dma_start(out=wt1, in_=w_d[:, 1])
            nc.sync.dma_start(out=xts[1], in_=x_d[:, 1])
            nc.sync.dma_start(out=xts[3], in_=x_d[:, 3])
            nc.scalar.dma_start(out=wt0, in_=w_d[:, 0])
            nc.scalar.dma_start(out=xt0z, in_=x_d[:, 0, X0Z_BASE:])
            nc.scalar.dma_start(out=xts[2], in_=x_d[:, 2])
            # The duplicate loads are pure overhead for the HBM-bound head
            # window, so gate each on the LAST even load having landed (tiny
            # DVE copy reading xts[3]'s tail into the dup's slot range, so
            # the tracker orders copy -> DMA): they stream ~18-26us, well
            # before first use at ~33us.
            for n in (1, 2, 3):
                nc.vector.tensor_copy(
                    out=xto[n][:, 1:2, :], in_=xts[3][:, PIXP - 1 : PIXP, :]
                )
            nc.sync.dma_start(out=xto[1][:, 1 : 1 + PIXP], in_=x_d[:, 1])
            nc.scalar.dma_start(out=xto[2][:, 1 : 1 + PIXP], in_=x_d[:, 2])
            nc.sync.dma_start(out=xto[3][:, 1 : 1 + PIXP], in_=x_d[:, 3])

            def x_src(n, t):
                """(x tile, pixel base) holding rows needed by row tile t."""
                if n == 0:
                    if t < 2:
                        return xt0a, 0
                    if t < 4:
                        return xt0m, X0M_BASE
                    return xt0z, X0Z_BASE
                return xts[n], 0

            def w_sel(oc, k):
                """Stationary weight AP [128, 2, 128] for (oc, tap k)."""
                return (wt0 if oc == 0 else wt1)[:, k]

            # Last block gets a small trailing row-tile so the final
            # PSUM-evacuate -> store -> HBM-write-receipt chain (which the
            # exit barrier serializes on) covers only 4 rows.
            TILES = [(t * ROWS_PER_TILE, ROWS_PER_TILE) for t in range(N_ROWTILES)]
            TILES_LAST = TILES[:5] + [(45, 5), (50, 4)]

            for n in range(IMGS):
                for oc in range(2):
                    last_block = n == IMGS - 1 and oc == 1
                    tiles = TILES_LAST if last_block else TILES
                    # staging for a full (n, oc) output block: dense 54x54
                    # rows so stores move 5.8KB-contiguous lines/partition.
                    ot = out_pool.tile([128, OH * OW], dt_out)
                    for t, (h0, rows) in enumerate(tiles):
                        nt_in = rows * W
                        nt_out = rows * OW
                        ob = h0 * OW  # fp16 staging offset of this tile
                        xsrc, xbase = x_src(n, min(t, 5))
                        pt = psum_pool.tile([128, nt_out], mybir.dt.float32)
                        k = 0
                        for kh in range(3):
                            for kw in range(3):
                                # kw=1 taps on images 1-3 read the shifted
                                # copy so their base byte is 4B-aligned
                                if n >= 1 and kw == 1:
                                    xs, xb = xto[n], -1
                                else:
                                    xs, xb = xsrc, xbase
                                off = (h0 + kh) * W + kw - xb
                                # strided moving AP skips the 2 junk cols per
                                # row: [128, 2 chunks (stride 1B), rows
                                # (stride 112B), 54 cols (stride 2B)] — base
                                # byte 2*off is always even.
                                if mode == "fp8dr":
                                    rhs = xs[:, off : off + nt_in, :].rearrange(
                                        "p (r q) c -> p c r q", q=W
                                    )[:, :, :, 0:OW]
                                    nc.tensor.matmul(
                                        pt,
                                        w_sel(oc, k),
                                        rhs,
                                        start=(k == 0),
                                        stop=(k == 8),
                                        perf_mode=mybir.MatmulPerfMode.DoubleRow,
                                    )
                                else:
                                    for c in range(2):
                                        rhs = xs[:, off : off + nt_in, c].rearrange(
                                            "p (r q) -> p r q", q=W
                                        )[:, :, 0:OW]
                                        nc.tensor.matmul(
                                            pt,
                                            w_sel(oc, k)[:, c],
                                            rhs,
                                            start=(k == 0 and c == 0),
                                            stop=(k == 8 and c == 1),
                                        )
                                k += 1
                        nc.vector.tensor_copy(
                            out=ot[:, ob : ob + nt_out], in_=pt
                        )
                        if last_block:
                            # fine-grained stores on the final block: pairs
                            # early, singles at the end so the final store
                            # (and its HBM write receipt, which the exit
                            # barrier waits on) covers only 4 rows. The
                            # 5-row and 4-row pieces go on different rings.
                            if t in (1, 3):
                                nc.sync.dma_start(
                                    out=out_d[n, oc * 128 : (oc + 1) * 128,
                                              h0 - ROWS_PER_TILE : h0 + rows, :],
                                    in_=ot[:, ob - ROWS_PER_TILE * OW : ob + nt_out]
                                    .rearrange("p (h w) -> p h w", w=OW),
                                )
                            elif t >= 4:
                                ring = nc.scalar if t == len(tiles) - 1 else nc.sync
                                ring.dma_start(
                                    out=out_d[n, oc * 128 : (oc + 1) * 128,
                                              h0 : h0 + rows, :],
                                    in_=ot[:, ob : ob + nt_out].rearrange(
                                        "p (h w) -> p h w", w=OW
                                    ),
                                )
                    if not last_block:
                        nc.sync.dma_start(
                            out=out_d[n, oc * 128 : (oc + 1) * 128, :, :],
                            in_=ot.rearrange("p (h w) -> p h w", w=OW),
                        )
    nc.compile()
    return nc


def get_program(mode="fp8dr"):
    if mode not in _PROGRAM_CACHE:
        _PROGRAM_CACHE[mode] = _build_program(mode)
    return _PROGRAM_CACHE[mode]


def _np_dtype(mode):
    return ml_dtypes.float8_e4m3 if mode == "fp8dr" else ml_dtypes.bfloat16


def prep_weight(weight, mode="fp8dr"):
    """weight [256, 256, 3, 3] OIHW fp32 -> w_sb [128 ki, 2 oc, 9 tap, 2 c, 128 m]."""
    wq = weight.astype(np.int32).astype(np.float32)
    wq = wq.reshape(2, 128, 2, 128, 3, 3)  # [oc, m, c, ki, kh, kw]
    w_sb = np.ascontiguousarray(wq.transpose(3, 0, 4, 5, 2, 1))  # [ki, oc, kh, kw, c, m]
    w_sb = w_sb.reshape(128, 2, 9, 2, 128)
    return w_sb.astype(_np_dtype(mode))


def prep_x_core(x_core, mode="fp8dr"):
    """x_core [IMGS, 256, 56, 56] int32 -> x_sb [128 ki, IMGS, PIXP, 2 c]."""
    xq = np.clip(x_core.astype(np.int32), 0, 7).astype(np.float32)
    xq = xq.reshape(IMGS, 2, 128, PIX)  # [n, c, ki, pix]
    x_sb = np.zeros((128, IMGS, PIXP, 2), np.float32)
    x_sb[:, :, :PIX, :] = xq.transpose(2, 0, 3, 1)
    return x_sb.astype(_np_dtype(mode))


def make_in_maps(x, weight, mode="fp8dr"):
    w_sb = prep_weight(weight, mode)
    return [
        {"x_sb": prep_x_core(x[c * IMGS : (c + 1) * IMGS], mode), "w_sb": w_sb}
        for c in range(N_CORES)
    ]


def kernel(x, weight):
    import time

    from concourse.bass_utils import run_bass_kernel_spmd

    mode = "fp8dr"
    nc = get_program(mode)
    in_maps = make_in_maps(np.asarray(x), np.asarray(weight), mode)
    last_err = None
    for attempt in range(3):
        try:
            res = run_bass_kernel_spmd(nc, in_maps, list(range(N_CORES)))
            break
        except Exception as e:  # transient NRT_EXEC_UNIT_UNRECOVERABLE flakes
            last_err = e
            time.sleep(2.0)
    else:
        raise last_err
    return np.concatenate(
        [res.results[c]["out"] for c in range(N_CORES)], axis=0
    ).astype(np.float32)
